# revision 7
# baseline (speedup 1.0000x reference)
"""Trainium2 Bass kernel for 7x7 valid cross-correlation on a 4096x4096 image.

Strategy: shard output rows across 8 NeuronCores (512 rows/core). Each core
receives its input row-slab WITH the (kh-1)=6 halo rows already included, so
no device-side halo exchange is needed. On-core, the conv is computed on the
tensor engine as 7 PSUM-accumulating matmuls per output tile: for each kernel
column dx, a banded-Toeplitz stationary matrix B_dx[k, m] = w[k-m, dx]
contracts over 128 input rows while the moving operand is a column-shifted
view X[:, c0+dx : c0+dx+N] of the input slab already in SBUF.

Every matmul is the same verified shape (K=128, M=122, N=512, fp32r): the
last row/column tiles overlap their predecessors and only the fresh rows
are written out (fp32r gives wrong results for partial K or odd N).
"""

import numpy as np

H, W = 4096, 4096
KH, KW = 7, 7
N_CORES = 8
OH, OW = H - KH + 1, W - KW + 1          # 4090, 4090
RPC = H // N_CORES                        # 512 output rows per core (core 7: 506 valid)
IN_ROWS = RPC + KH - 1                    # 518 input rows per core
MT = 122                                  # output rows per row tile (128 - 6)
# (input/output row offset within slab, rows of outt to emit: [emit0, 122))
ROW_TILES = [(0, 0), (122, 0), (244, 0), (366, 0), (390, 98)]
CT_N = 512
# column tile output starts; last overlaps so every matmul has N=512
COL_STARTS = [0, 512, 1024, 1536, 2048, 2560, 3072, 3578]

# fp32r = relaxed-precision fp32 matmul (TF32-like): 1 cycle/row vs 4 for fp32.
USE_FP32R = True

_cache = {}


def _build_program(repeat=1):
    import concourse.bacc as bacc
    import concourse.mybir as mybir
    import concourse.tile as tile

    mm_dt = mybir.dt.float32r if USE_FP32R else mybir.dt.float32
    f32 = mybir.dt.float32

    nc = bacc.Bacc("TRN2", target_bir_lowering=False, debug=False,
                   num_devices=N_CORES)
    x = nc.dram_tensor("x", [IN_ROWS, W], mm_dt, kind="ExternalInput")
    bands = nc.dram_tensor("bands", [128, KW, MT], mm_dt, kind="ExternalInput")
    biasb = nc.dram_tensor("biasb", [128, 1], f32, kind="ExternalInput")
    y = nc.dram_tensor("y", [RPC, OW], f32, kind="ExternalOutput")

    with tile.TileContext(nc) as tc:
        with (
            tc.tile_pool(name="const", bufs=1) as cpool,
            tc.tile_pool(name="xs", bufs=3) as xpool,
            tc.tile_pool(name="out", bufs=2) as opool,
            tc.tile_pool(name="ps", bufs=8, space="PSUM") as pspool,
        ):
            bands_t = cpool.tile([128, KW, MT], mm_dt)
            nc.sync.dma_start(bands_t[:], bands[:])
            bias_t = cpool.tile([128, 1], f32)
            nc.sync.dma_start(bias_t[:], biasb[:])

            # The first slab is DMAed in column chunks so the first column
            # tiles' matmuls start before the whole 2MB lands; later slabs
            # prefetch far enough ahead that one full-size (more efficient)
            # DMA each is better.
            first_chunks = [(0, 1030), (1024, 1030), (2048, 1030),
                            (3072, 1024)]

            for rep in range(repeat):
                for it, (r0, emit0) in enumerate(ROW_TILES):
                    xs = xpool.tile([128, W], mm_dt, tag="xs")
                    chunks = first_chunks if (rep == 0 and it == 0) \
                        else [(0, W)]
                    for cc0, cw in chunks:
                        # scalar-engine HWDGE ring: runs parallel to the
                        # const/output DMAs on the sync-engine ring
                        nc.scalar.dma_start(xs[:, cc0:cc0 + cw],
                                            x[r0:r0 + 128, cc0:cc0 + cw])
                    outt = opool.tile([128, OW], f32, tag="out")
                    for c0 in COL_STARTS:
                        ps = pspool.tile([128, CT_N], f32, tag="ps")
                        for dx in range(KW):
                            nc.tensor.matmul(
                                ps[:MT, :],
                                bands_t[:, dx, :],
                                xs[:, c0 + dx: c0 + dx + CT_N],
                                start=(dx == 0),
                                stop=(dx == KW - 1),
                            )
                        nc.vector.tensor_scalar_add(
                            outt[:MT, c0:c0 + CT_N], ps[:MT, :],
                            bias_t[:MT, 0:1])
                    nc.sync.dma_start(
                        y[r0 + emit0: r0 + MT, :], outt[emit0:MT, :])

    nc.compile()
    return nc


def _get_program():
    if "nc" not in _cache:
        _cache["nc"] = _build_program()
    return _cache["nc"]


def _shard_inputs(X, weight, bias):
    X = np.ascontiguousarray(np.asarray(X, dtype=np.float32))
    weight = np.asarray(weight, dtype=np.float32)
    bias = np.asarray(bias, dtype=np.float32)

    # Host-side sharding: per-core input slab with halo rows (zero-padded at
    # the bottom edge for the last core; those output rows are discarded).
    slabs = np.zeros((N_CORES, IN_ROWS, W), dtype=np.float32)
    for i in range(N_CORES):
        r0 = RPC * i
        r1 = min(r0 + IN_ROWS, H)
        slabs[i, : r1 - r0] = X[r0:r1]

    # Banded-Toeplitz stationary matrices: bands[k, dx, m] = w[k-m, dx].
    bands = np.zeros((128, KW, MT), dtype=np.float32)
    for dy in range(KH):
        for m in range(MT):
            bands[m + dy, :, m] = weight[dy, :]

    biasb = np.broadcast_to(bias.reshape(1, 1), (128, 1)).copy()

    return [{"x": slabs[i], "bands": bands, "biasb": biasb}
            for i in range(N_CORES)]


def kernel(X, weight, bias):
    from concourse.bass_utils import run_bass_kernel_spmd

    nc = _get_program()
    in_maps = _shard_inputs(X, weight, bias)
    res = run_bass_kernel_spmd(nc, in_maps, list(range(N_CORES)))

    out = np.empty((OH, OW), dtype=np.float32)
    for i in range(N_CORES):
        r0 = RPC * i
        nrows = min(RPC, OH - r0)
        out[r0:r0 + nrows] = res.results[i]["y"][:nrows]
    return out


# revision 8
# speedup vs baseline: 1.2448x; 1.2448x over previous
"""Trainium2 Bass kernel for 7x7 valid cross-correlation on a 4096x4096 image.

Strategy: shard output rows across 8 NeuronCores (512 rows/core). Each core
receives its input row-slab WITH the (kh-1)=6 halo rows already included, so
no device-side halo exchange is needed. On-core, the conv is computed on the
tensor engine as 7 PSUM-accumulating matmuls per output tile: for each kernel
column dx, a banded-Toeplitz stationary matrix B_dx[k, m] = w[k-m, dx]
contracts over 128 input rows while the moving operand is a column-shifted
view X[:, c0+dx : c0+dx+N] of the input slab already in SBUF.

Every matmul is the same verified shape (K=128, M=122, N=512, fp32r): the
last row/column tiles overlap their predecessors and only the fresh rows
are written out (fp32r gives wrong results for partial K or odd N).
"""

import numpy as np

H, W = 4096, 4096
KH, KW = 7, 7
N_CORES = 8
OH, OW = H - KH + 1, W - KW + 1          # 4090, 4090
RPC = H // N_CORES                        # 512 output rows per core (core 7: 506 valid)
IN_ROWS = RPC + KH - 1                    # 518 input rows per core
MT = 122                                  # output rows per row tile (128 - 6)
# (input/output row offset within slab, rows of outt to emit: [emit0, 122))
ROW_TILES = [(0, 0), (122, 0), (244, 0), (366, 0), (390, 98)]
CT_N = 512
# column tile output starts; last overlaps so every matmul has N=512
COL_STARTS = [0, 512, 1024, 1536, 2048, 2560, 3072, 3578]

# fp32r = relaxed-precision fp32 matmul (TF32-like): 1 cycle/row vs 4 for fp32.
USE_FP32R = True

_cache = {}


def _build_program(repeat=1):
    import concourse.bacc as bacc
    import concourse.mybir as mybir
    import concourse.tile as tile

    mm_dt = mybir.dt.float32r if USE_FP32R else mybir.dt.float32
    f32 = mybir.dt.float32

    nc = bacc.Bacc("TRN2", target_bir_lowering=False, debug=False,
                   num_devices=N_CORES)
    x = nc.dram_tensor("x", [IN_ROWS, W], mm_dt, kind="ExternalInput")
    bands = nc.dram_tensor("bands", [128, KW, MT], mm_dt, kind="ExternalInput")
    biasb = nc.dram_tensor("biasb", [128, 1], f32, kind="ExternalInput")
    y = nc.dram_tensor("y", [RPC, OW], f32, kind="ExternalOutput")

    with tile.TileContext(nc) as tc:
        with (
            tc.tile_pool(name="const", bufs=1) as cpool,
            tc.tile_pool(name="xs", bufs=3) as xpool,
            tc.tile_pool(name="out", bufs=2) as opool,
            tc.tile_pool(name="ps", bufs=8, space="PSUM") as pspool,
        ):
            bands_t = cpool.tile([128, KW, MT], mm_dt)
            nc.sync.dma_start(bands_t[:], bands[:])
            bias_t = cpool.tile([128, 1], f32)
            nc.sync.dma_start(bias_t[:], biasb[:])

            # The first slab is DMAed in column chunks so the first column
            # tiles' matmuls start before the whole 2MB lands; later slabs
            # prefetch far enough ahead that one full-size (more efficient)
            # DMA each is better.
            first_chunks = [(0, 1030), (1024, 1030), (2048, 1030),
                            (3072, 1024)]

            for rep in range(repeat):
                for it, (r0, emit0) in enumerate(ROW_TILES):
                    xs = xpool.tile([128, W], mm_dt, tag="xs")
                    chunks = first_chunks
                    for cc0, cw in chunks:
                        # scalar-engine HWDGE ring: runs parallel to the
                        # const/output DMAs on the sync-engine ring
                        nc.scalar.dma_start(xs[:, cc0:cc0 + cw],
                                            x[r0:r0 + 128, cc0:cc0 + cw])
                    outt = opool.tile([128, OW], f32, tag="out")
                    for c0 in COL_STARTS:
                        ps = pspool.tile([128, CT_N], f32, tag="ps")
                        for dx in range(KW):
                            nc.tensor.matmul(
                                ps[:MT, :],
                                bands_t[:, dx, :],
                                xs[:, c0 + dx: c0 + dx + CT_N],
                                start=(dx == 0),
                                stop=(dx == KW - 1),
                            )
                        nc.vector.tensor_scalar_add(
                            outt[:MT, c0:c0 + CT_N], ps[:MT, :],
                            bias_t[:MT, 0:1])
                    nc.sync.dma_start(
                        y[r0 + emit0: r0 + MT, :], outt[emit0:MT, :])

    nc.compile()
    return nc


def _get_program():
    if "nc" not in _cache:
        _cache["nc"] = _build_program()
    return _cache["nc"]


def _shard_inputs(X, weight, bias):
    X = np.ascontiguousarray(np.asarray(X, dtype=np.float32))
    weight = np.asarray(weight, dtype=np.float32)
    bias = np.asarray(bias, dtype=np.float32)

    # Host-side sharding: per-core input slab with halo rows (zero-padded at
    # the bottom edge for the last core; those output rows are discarded).
    slabs = np.zeros((N_CORES, IN_ROWS, W), dtype=np.float32)
    for i in range(N_CORES):
        r0 = RPC * i
        r1 = min(r0 + IN_ROWS, H)
        slabs[i, : r1 - r0] = X[r0:r1]

    # Banded-Toeplitz stationary matrices: bands[k, dx, m] = w[k-m, dx].
    bands = np.zeros((128, KW, MT), dtype=np.float32)
    for dy in range(KH):
        for m in range(MT):
            bands[m + dy, :, m] = weight[dy, :]

    biasb = np.broadcast_to(bias.reshape(1, 1), (128, 1)).copy()

    return [{"x": slabs[i], "bands": bands, "biasb": biasb}
            for i in range(N_CORES)]


def kernel(X, weight, bias):
    from concourse.bass_utils import run_bass_kernel_spmd

    nc = _get_program()
    in_maps = _shard_inputs(X, weight, bias)
    res = run_bass_kernel_spmd(nc, in_maps, list(range(N_CORES)))

    out = np.empty((OH, OW), dtype=np.float32)
    for i in range(N_CORES):
        r0 = RPC * i
        nrows = min(RPC, OH - r0)
        out[r0:r0 + nrows] = res.results[i]["y"][:nrows]
    return out


# revision 9
# speedup vs baseline: 1.4485x; 1.1636x over previous
"""Trainium2 Bass kernel for 7x7 valid cross-correlation on a 4096x4096 image.

Strategy: shard output rows across 8 NeuronCores (512 rows/core). Each core
receives its input row-slab WITH the (kh-1)=6 halo rows already included, so
no device-side halo exchange is needed. On-core, the conv is computed on the
tensor engine as 7 PSUM-accumulating matmuls per output tile: for each kernel
column dx, a banded-Toeplitz stationary matrix B_dx[k, m] = w[k-m, dx]
contracts over 128 input rows while the moving operand is a column-shifted
view X[:, c0+dx : c0+dx+N] of the input slab already in SBUF.

Every matmul is the same verified shape (K=128, M=122, N=512, fp32r): the
last row/column tiles overlap their predecessors and only the fresh rows
are written out (fp32r gives wrong results for partial K or odd N).
"""

import numpy as np

H, W = 4096, 4096
KH, KW = 7, 7
N_CORES = 8
OH, OW = H - KH + 1, W - KW + 1          # 4090, 4090
RPC = H // N_CORES                        # 512 output rows per core (core 7: 506 valid)
IN_ROWS = RPC + KH - 1                    # 518 input rows per core
MT = 122                                  # output rows per row tile (128 - 6)
# (input/output row offset within slab, rows of outt to emit: [emit0, 122))
ROW_TILES = [(0, 0), (122, 0), (244, 0), (366, 0), (390, 98)]
CT_N = 512
# column tile output starts; last overlaps so every matmul has N=512
COL_STARTS = [0, 512, 1024, 1536, 2048, 2560, 3072, 3578]

# fp32r = relaxed-precision fp32 matmul (TF32-like): 1 cycle/row vs 4 for fp32.
USE_FP32R = True

_cache = {}


def _build_program(repeat=1):
    import concourse.bacc as bacc
    import concourse.mybir as mybir
    import concourse.tile as tile

    mm_dt = mybir.dt.float32r if USE_FP32R else mybir.dt.float32
    f32 = mybir.dt.float32

    nc = bacc.Bacc("TRN2", target_bir_lowering=False, debug=False,
                   num_devices=N_CORES)
    x = nc.dram_tensor("x", [IN_ROWS, W], mm_dt, kind="ExternalInput")
    bands = nc.dram_tensor("bands", [128, KW, MT], mm_dt, kind="ExternalInput")
    biasb = nc.dram_tensor("biasb", [128, 1], f32, kind="ExternalInput")
    y = nc.dram_tensor("y", [RPC, OW], f32, kind="ExternalOutput")

    with tile.TileContext(nc) as tc:
        with (
            tc.tile_pool(name="const", bufs=1) as cpool,
            tc.tile_pool(name="xs", bufs=4) as xpool,
            tc.tile_pool(name="out", bufs=3) as opool,
            tc.tile_pool(name="ps", bufs=8, space="PSUM") as pspool,
        ):
            bands_t = cpool.tile([128, KW, MT], mm_dt)
            nc.sync.dma_start(bands_t[:], bands[:])
            bias_t = cpool.tile([128, 1], f32)
            nc.sync.dma_start(bias_t[:], biasb[:])

            # The first slab is DMAed in column chunks so the first column
            # tiles' matmuls start before the whole 2MB lands; later slabs
            # prefetch far enough ahead that one full-size (more efficient)
            # DMA each is better.
            first_chunks = [(c, min(518, 4096 - c))
                            for c in [0, 512, 1024, 1536, 2048, 2560,
                                      3072, 3578]]

            for rep in range(repeat):
                for it, (r0, emit0) in enumerate(ROW_TILES):
                    xs = xpool.tile([128, W], mm_dt, tag="xs")
                    chunks = first_chunks
                    for cc0, cw in chunks:
                        # scalar-engine HWDGE ring: runs parallel to the
                        # const/output DMAs on the sync-engine ring
                        nc.scalar.dma_start(xs[:, cc0:cc0 + cw],
                                            x[r0:r0 + 128, cc0:cc0 + cw])
                    outt = opool.tile([128, OW], f32, tag="out")
                    for c0 in COL_STARTS:
                        ps = pspool.tile([128, CT_N], f32, tag="ps")
                        for dx in range(KW):
                            nc.tensor.matmul(
                                ps[:MT, :],
                                bands_t[:, dx, :],
                                xs[:, c0 + dx: c0 + dx + CT_N],
                                start=(dx == 0),
                                stop=(dx == KW - 1),
                            )
                        nc.vector.tensor_scalar_add(
                            outt[:MT, c0:c0 + CT_N], ps[:MT, :],
                            bias_t[:MT, 0:1])
                    nc.sync.dma_start(
                        y[r0 + emit0: r0 + MT, :], outt[emit0:MT, :])

    nc.compile()
    return nc


def _get_program():
    if "nc" not in _cache:
        _cache["nc"] = _build_program()
    return _cache["nc"]


def _shard_inputs(X, weight, bias):
    X = np.ascontiguousarray(np.asarray(X, dtype=np.float32))
    weight = np.asarray(weight, dtype=np.float32)
    bias = np.asarray(bias, dtype=np.float32)

    # Host-side sharding: per-core input slab with halo rows (zero-padded at
    # the bottom edge for the last core; those output rows are discarded).
    slabs = np.zeros((N_CORES, IN_ROWS, W), dtype=np.float32)
    for i in range(N_CORES):
        r0 = RPC * i
        r1 = min(r0 + IN_ROWS, H)
        slabs[i, : r1 - r0] = X[r0:r1]

    # Banded-Toeplitz stationary matrices: bands[k, dx, m] = w[k-m, dx].
    bands = np.zeros((128, KW, MT), dtype=np.float32)
    for dy in range(KH):
        for m in range(MT):
            bands[m + dy, :, m] = weight[dy, :]

    biasb = np.broadcast_to(bias.reshape(1, 1), (128, 1)).copy()

    return [{"x": slabs[i], "bands": bands, "biasb": biasb}
            for i in range(N_CORES)]


def kernel(X, weight, bias):
    from concourse.bass_utils import run_bass_kernel_spmd

    nc = _get_program()
    in_maps = _shard_inputs(X, weight, bias)
    res = run_bass_kernel_spmd(nc, in_maps, list(range(N_CORES)))

    out = np.empty((OH, OW), dtype=np.float32)
    for i in range(N_CORES):
        r0 = RPC * i
        nrows = min(RPC, OH - r0)
        out[r0:r0 + nrows] = res.results[i]["y"][:nrows]
    return out
